# revision 17
# baseline (speedup 1.0000x reference)
"""Trainium2 Bass kernel for nn_AttentionQKV (causal attention + GCN refinement).

Sharding: batch*heads across 8 cores (core c: batch c//4, heads [4*(c%4), 4*(c%4)+4)).
Output: per-pair AllToAll of head outputs (token-sharded), then each core computes
the FULL Wo contraction + output layernorm for its 512-token slice.

Per-core math (head h, E = exp(QK^T/sqrt(d)) causal, D_i = sum_j E_ij):
  attn = D^-1 E;  adj = attn with diag set to 1;  deg_i = 2 - attn_ii
  (adj@h)_i/deg_i = a_i (Eh)_i + b_i h_i  with  a = 1/(2D - eii), b = (D-eii)*a
  E@h1 = (E@V)@W1 (layer-1 linearity);  D free as 65th ones-column of [V|1].
Per-token scalars (invd=1/D, a, b) are computed in [token%128, tile] column layout
(full 128 DVE lanes, trivial cost) via a DRAM round-trip, then materialized as
[64, N] broadcast matrices by step-0 DMA reads (a, b) or PE rank-1 matmul (invd).
GNN matmuls act on head-PAIR tiles [128, N] with block-diag(W, W) stationaries so
one matmul covers two heads, and elementwise/gelu work uses all 128 partitions.
"""

import os
import numpy as np

import concourse.bass as bass
import concourse.tile as tile
import concourse.mybir as mybir
from concourse import bacc
from concourse.bass_utils import run_bass_kernel_spmd
from concourse.masks import make_identity

dt = mybir.dt
F32 = dt.float32
F32R = dt.float32r
BF16 = dt.bfloat16
AF = mybir.ActivationFunctionType
ALU = mybir.AluOpType

B, N, DIM = 2, 2048, 1024
HEADS, DHEAD = 16, 64
HPC = 4                 # heads per core
EPS = 1e-5
NT = N // 128           # 16 token tiles
KT8 = DIM // 128        # 8 dim tiles
SCALE = DHEAD ** -0.5

# column offset of j-tile jt inside the packed E^T tile (width of jt = N-128*jt)
ET_OFF = [0]
for _jt in range(1, NT):
    ET_OFF.append(ET_OFF[-1] + (N - 128 * (_jt - 1)))
ET_TOT = ET_OFF[-1] + (N - 128 * (NT - 1))  # 17408


def _bc(ap, parts):
    """broadcast a DRAM AP across `parts` partitions (prepend step-0 dim)."""
    dims = list(ap.ap)
    if dims and dims[0][1] == 1:
        dims = dims[1:]
    return bass.AP(tensor=ap.tensor, offset=ap.offset, ap=[[0, parts]] + dims)


def build_program(has_gnn_b, has_lin_b, gin_ones, gout_ones):
    nc = bacc.Bacc("TRN2", target_bir_lowering=False, debug=False, num_devices=8)

    x_in = nc.dram_tensor("x_in", [N, DIM], F32, kind="ExternalInput").ap()
    wq_in = nc.dram_tensor("wq_in", [DIM, 256], F32R, kind="ExternalInput").ap()
    wk_in = nc.dram_tensor("wk_in", [DIM, 256], F32R, kind="ExternalInput").ap()
    wv_in = nc.dram_tensor("wv_in", [DIM, 256], F32R, kind="ExternalInput").ap()
    wo_in = nc.dram_tensor("wo_in", [DIM, DIM], BF16, kind="ExternalInput").ap()
    gin_in = nc.dram_tensor("gin_in", [1, DIM], F32, kind="ExternalInput").ap()
    gout_in = nc.dram_tensor("gout_in", [1, DIM], F32, kind="ExternalInput").ap()
    gnw_in = nc.dram_tensor("gnw_in", [2, DHEAD, DHEAD], BF16, kind="ExternalInput").ap()
    gnb_in = nc.dram_tensor("gnb_in", [2, DHEAD], F32, kind="ExternalInput").ap()
    linw_in = nc.dram_tensor("linw_in", [DHEAD, DHEAD], BF16, kind="ExternalInput").ap()
    linb_in = nc.dram_tensor("linb_in", [1, DHEAD], F32, kind="ExternalInput").ap()
    bmask_in = nc.dram_tensor("bmask_in", [1, 2], F32, kind="ExternalInput").ap()
    y_out = nc.dram_tensor("y_out", [512, DIM], F32, kind="ExternalOutput").ap()

    # DRAM scratch: per-head row staging (D, Sii), per-head scalar rows
    rows_d = nc.dram_tensor("rows_d", [HPC, 2, N], F32).ap()     # D, Sii (raw)
    rows_qf = nc.dram_tensor("rows_qf", [HPC, N], F32).ap()      # invd
    rows_qb = nc.dram_tensor("rows_qb", [HPC, 2, N], BF16).ap()  # a, b
    rows_da = nc.dram_tensor("rows_da", [HPC, N], BF16).ap() if has_gnn_b else None
    cc_in = [nc.dram_tensor(f"cc_in{k}", [1024, 512], BF16).ap() for k in range(2)]
    cc_out = [nc.dram_tensor(f"cc_out{k}", [1024, 512], BF16).ap() for k in range(2)]

    # single 8-rank group (4-rank mesh A2A unsupported); each sender duplicates
    # its 4 token-chunks into both batch halves, receivers mask-combine with
    # the per-core bmask input.
    groups = [[0, 1, 2, 3, 4, 5, 6, 7]]

    with tile.TileContext(nc) as tc:
        from contextlib import ExitStack
        with ExitStack() as ctx:
            const = ctx.enter_context(tc.tile_pool(name="const", bufs=1))
            persist = ctx.enter_context(tc.tile_pool(name="persist", bufs=1))

            ident32 = const.tile([128, 128], F32)
            make_identity(nc, ident32[:])
            ident_r = const.tile([128, 128], F32R)
            nc.vector.tensor_copy(out=ident_r[:], in_=ident32[:])
            ident_b = const.tile([128, 128], BF16)
            nc.vector.tensor_copy(out=ident_b[:], in_=ident32[:])
            ones_r = const.tile([128, 1], F32R)
            ones32 = const.tile([128, 1], F32)
            nc.vector.memset(ones32[:], 1.0)
            nc.vector.tensor_copy(out=ones_r[:], in_=ones32[:])
            eps_t = const.tile([128, 1], F32)
            nc.vector.memset(eps_t[:], EPS)
            bm = const.tile([128, 2], F32)
            nc.sync.dma_start(out=bm[:], in_=_bc(bmask_in, 128))
            # rank-1 pair-broadcast stationaries: row of 1s over cols 0-63 / 64-127
            ones2a = const.tile([1, 128], F32R)
            ones2b = const.tile([1, 128], F32R)
            ones2f = const.tile([1, 2, 128], F32)
            nc.vector.memset(ones2f[:], 0.0)
            nc.vector.memset(ones2f[0:1, 0, 0:64], 1.0)
            nc.vector.memset(ones2f[0:1, 1, 64:128], 1.0)
            nc.vector.tensor_copy(out=ones2a[:], in_=ones2f[0:1, 0, :])
            nc.vector.tensor_copy(out=ones2b[:], in_=ones2f[0:1, 1, :])

            # block-diag GNN stationaries [128, 128]: diag(W, W)
            w1bd = const.tile([128, 128], BF16)
            w2bd = const.tile([128, 128], BF16)
            linbd = const.tile([128, 128], BF16)
            for t in (w1bd, w2bd, linbd):
                nc.vector.memset(t[:], 0.0)
            for hh in range(2):
                sl = slice(hh * 64, hh * 64 + 64)
                nc.sync.dma_start(out=w1bd[sl, sl], in_=gnw_in[0])
                nc.sync.dma_start(out=w2bd[sl, sl], in_=gnw_in[1])
                nc.sync.dma_start(out=linbd[sl, sl], in_=linw_in)
            gnb_sb = linb_sb = None
            if has_gnn_b:
                gnb_sb = const.tile([128, 2], F32)   # stacked pair bias cols (d, l)
                for hh in range(2):
                    nc.sync.dma_start(
                        out=gnb_sb[hh * 64:hh * 64 + 64, :],
                        in_=bass.AP(tensor=gnb_in.tensor, offset=gnb_in.offset,
                                    ap=[[1, DHEAD], [DHEAD, 2]]))
            if has_lin_b:
                linb_sb = const.tile([128, 1], F32)
                for hh in range(2):
                    nc.sync.dma_start(
                        out=linb_sb[hh * 64:hh * 64 + 64, :],
                        in_=bass.AP(tensor=linb_in.tensor, offset=linb_in.offset,
                                    ap=[[1, DHEAD], [DHEAD, 1]]))
            gin_col = None
            if not gin_ones:
                gin_col = const.tile([128, KT8], F32)
                nc.sync.dma_start(out=gin_col[:], in_=bass.AP(
                    tensor=gin_in.tensor, offset=gin_in.offset, ap=[[1, 128], [128, KT8]]))
            gout_mat = None
            if not gout_ones:
                gout_mat = const.tile([128, DIM], F32)
                nc.sync.dma_start(out=gout_mat[:], in_=_bc(gout_in, 128))

            # persistent tensors
            qt = [persist.tile([128, N], F32R, name=f"qt{p}") for p in range(2)]
            kt = [persist.tile([128, N], F32R, name=f"kt{p}") for p in range(2)]
            vt1 = persist.tile([128, NT, HPC, 65], BF16)      # [V_h | 1] token layout

            # ---------- Phase 1+2: LN -> xnT -> QKV projections ----------
            with tc.tile_pool(name="proj", bufs=1) as proj, \
                 tc.tile_pool(name="ph1", bufs=4) as ph1, \
                 tc.tile_pool(name="ph1ps", bufs=2, space="PSUM") as ph1ps:
                wq_sb = proj.tile([128, KT8, 256], F32R, name="wq_sb")
                wk_sb = proj.tile([128, KT8, 256], F32R, name="wk_sb")
                wv_sb = proj.tile([128, KT8, 256], F32R, name="wv_sb")
                nc.sync.dma_start(out=wq_sb[:], in_=wq_in.rearrange("(k p) d -> p k d", p=128))
                nc.sync.dma_start(out=wk_sb[:], in_=wk_in.rearrange("(k p) d -> p k d", p=128))
                nc.sync.dma_start(out=wv_sb[:], in_=wv_in.rearrange("(k p) d -> p k d", p=128))
                xnT = proj.tile([128, KT8, N], F32R, name="xnT")

                for it in range(NT):
                    xt = ph1.tile([128, DIM], F32, name="xt")
                    nc.sync.dma_start(out=xt[:], in_=x_in[it * 128:(it + 1) * 128, :])
                    st = ph1.tile([128, 2, nc.vector.BN_STATS_DIM], F32, name="st")
                    for sg in range(2):
                        nc.vector.bn_stats(out=st[:, sg, :], in_=xt[:, sg * 512:(sg + 1) * 512])
                    mv = ph1.tile([128, nc.vector.BN_AGGR_DIM], F32, name="mv")
                    nc.vector.bn_aggr(out=mv[:], in_=st[:])
                    rstd = ph1.tile([128, 1], F32, name="rstd")
                    nc.scalar.activation(out=rstd[:], in_=mv[:, 1:2], func=AF.Sqrt, bias=eps_t[:])
                    nc.vector.reciprocal(out=rstd[:], in_=rstd[:])
                    nmr = ph1.tile([128, 1], F32, name="nmr")
                    nc.vector.tensor_scalar(out=nmr[:], in0=mv[:, 0:1], scalar1=rstd[:],
                                            scalar2=-1.0, op0=ALU.mult, op1=ALU.mult)
                    xnt = ph1.tile([128, DIM], F32R, name="xnt")
                    nc.scalar.activation(out=xnt[:], in_=xt[:], func=AF.Identity,
                                         bias=nmr[:], scale=rstd[:])
                    for half in range(2):
                        ps = ph1ps.tile([128, 512], F32R, name="trps")
                        for q in range(4):
                            d8 = half * 4 + q
                            nc.tensor.transpose(ps[:, q * 128:(q + 1) * 128],
                                                xnt[:, d8 * 128:(d8 + 1) * 128], ident_r[:])
                        for q in range(4):
                            d8 = half * 4 + q
                            dst = xnT[:, d8, it * 128:(it + 1) * 128]
                            src = ps[:, q * 128:(q + 1) * 128]
                            if q % 2 == 0:
                                if gin_ones:
                                    nc.scalar.copy(out=dst, in_=src.bitcast(F32))
                                else:
                                    nc.scalar.activation(out=dst, in_=src.bitcast(F32),
                                                         func=AF.Copy, scale=gin_col[:, d8:d8 + 1])
                            else:
                                if gin_ones:
                                    nc.vector.tensor_copy(out=dst, in_=src)
                                else:
                                    nc.vector.tensor_scalar_mul(out=dst, in0=src.bitcast(F32),
                                                                scalar1=gin_col[:, d8:d8 + 1])

                # QT/KT per head pair (QT pre-scaled by 1/sqrt(d))
                for p in range(2):
                    for nch in range(4):
                        sl = slice(nch * 512, (nch + 1) * 512)
                        psq = ph1ps.tile([128, 512], F32, name="psq")
                        psk = ph1ps.tile([128, 512], F32, name="psk")
                        for kk in range(KT8):
                            nc.tensor.matmul(psq[:], wq_sb[:, kk, p * 128:(p + 1) * 128],
                                             xnT[:, kk, sl], start=(kk == 0), stop=(kk == KT8 - 1))
                        for kk in range(KT8):
                            nc.tensor.matmul(psk[:], wk_sb[:, kk, p * 128:(p + 1) * 128],
                                             xnT[:, kk, sl], start=(kk == 0), stop=(kk == KT8 - 1))
                        nc.scalar.activation(out=qt[p][:, sl], in_=psq[:], func=AF.Copy, scale=SCALE)
                        nc.scalar.copy(out=kt[p][:, sl], in_=psk[:])
                # V (token layout, bf16, with ones column)
                for it in range(NT):
                    psv = ph1ps.tile([128, 256], F32, name="psv")
                    for kk in range(KT8):
                        nc.tensor.matmul(psv[:], xnT[:, kk, it * 128:(it + 1) * 128],
                                         wv_sb[:, kk, :], start=(kk == 0), stop=(kk == KT8 - 1))
                    nc.vector.tensor_copy(out=vt1[:, it, :, 0:64],
                                          in_=psv[:].rearrange("p (h d) -> p h d", h=HPC))
                nc.gpsimd.memset(vt1[:, :, :, 64:65], 1.0)

            # ---------- Phase 3: per-head attention + GNN (pair-packed) ----------
            with tc.tile_pool(name="etpool", bufs=2) as etp, \
                 tc.tile_pool(name="hw", bufs=1) as hw, \
                 tc.tile_pool(name="hw2", bufs=2) as hw2, \
                 tc.tile_pool(name="p3ps", bufs=2, space="PSUM") as p3ps, \
                 tc.tile_pool(name="smps", bufs=2, space="PSUM") as smps, \
                 tc.tile_pool(name="stps", bufs=2, space="PSUM") as stps:
                P = dict(etp=etp, hw=hw, hw2=hw2, p3ps=p3ps, smps=smps, stps=stps,
                         ones_r=ones_r, ones2a=ones2a, ones2b=ones2b, ident_b=ident_b,
                         w1bd=w1bd, w2bd=w2bd, linbd=linbd,
                         gnb_sb=gnb_sb, linb_sb=linb_sb,
                         has_gnn_b=has_gnn_b, has_lin_b=has_lin_b,
                         qt=qt, kt=kt, vt1=vt1,
                         rows_d=rows_d, rows_qf=rows_qf, rows_qb=rows_qb,
                         rows_da=rows_da, et={}, rst={}, qk={})

                _stage1(tc, nc, P, 0)
                _stage1(tc, nc, P, 1)
                g0 = _pair(tc, nc, P, 0, cc_in, cc_out, groups)
                next(g0)             # stage A (passA + scalar chain, both heads)
                _stage1(tc, nc, P, 2)
                next(g0)             # stage B (GNN layer 1 + h2 + passB(a))
                _stage1(tc, nc, P, 3)
                next(g0, None)       # stage C (passB(b) + finish + A2A 0)
                g1 = _pair(tc, nc, P, 1, cc_in, cc_out, groups)
                next(g1)
                next(g1)
                next(g1, None)

            # ---------- Phase 4: gather + full Wo + output LN ----------
            with tc.tile_pool(name="ph4", bufs=2) as ph4, \
                 tc.tile_pool(name="ph4c", bufs=1) as ph4c, \
                 tc.tile_pool(name="ph4ps", bufs=4, space="PSUM") as ph4ps:
                wo_sb = ph4c.tile([128, KT8, DIM], BF16, name="wo_sb")
                nc.sync.dma_start(out=wo_sb[:], in_=wo_in.rearrange("(k p) d -> p k d", p=128))
                wos = []
                for p in range(2):
                    wall = ph4c.tile([128, 8, 512], BF16, name=f"wall{p}")
                    nc.sync.dma_start(out=wall[:],
                                      in_=cc_out[p].rearrange("(s p) t -> p s t", p=128))
                    w = ph4c.tile([128, 4, 512], BF16, name=f"wos{p}")
                    for s in range(4):
                        nc.vector.tensor_scalar_mul(out=w[:, s, :], in0=wall[:, s, :],
                                                    scalar1=bm[:, 0:1])
                        nc.vector.scalar_tensor_tensor(
                            out=w[:, s, :], in0=wall[:, s + 4, :], scalar=bm[:, 1:2],
                            in1=w[:, s, :], op0=ALU.mult, op1=ALU.add)
                    wos.append(w)
                for tt in range(4):
                    pss = []
                    for ch in range(2):
                        ps = ph4ps.tile([128, 512], F32, name="ps4")
                        for kt8 in range(KT8):
                            s, q = divmod(kt8, 2)
                            nc.tensor.matmul(ps[:], wos[q][:, s, tt * 128:(tt + 1) * 128],
                                             wo_sb[:, kt8, ch * 512:(ch + 1) * 512],
                                             start=(kt8 == 0), stop=(kt8 == KT8 - 1))
                        pss.append(ps)
                    st = ph4.tile([128, 2, nc.vector.BN_STATS_DIM], F32, name="st4")
                    for sg in range(2):
                        nc.vector.bn_stats(out=st[:, sg, :], in_=pss[sg][:])
                    mv = ph4.tile([128, nc.vector.BN_AGGR_DIM], F32, name="mv4")
                    nc.vector.bn_aggr(out=mv[:], in_=st[:])
                    rstd = ph4.tile([128, 1], F32, name="rstd4")
                    nc.scalar.activation(out=rstd[:], in_=mv[:, 1:2], func=AF.Sqrt, bias=eps_t[:])
                    nc.vector.reciprocal(out=rstd[:], in_=rstd[:])
                    ot_t = ph4.tile([128, DIM], F32, name="ot_t")
                    for sg in range(2):
                        nc.vector.tensor_scalar(out=ot_t[:, sg * 512:(sg + 1) * 512],
                                                in0=pss[sg][:], scalar1=mv[:, 0:1],
                                                scalar2=rstd[:], op0=ALU.subtract, op1=ALU.mult)
                    if not gout_ones:
                        nc.vector.tensor_tensor(out=ot_t[:], in0=ot_t[:], in1=gout_mat[:],
                                                op=ALU.mult)
                    nc.sync.dma_start(out=y_out[tt * 128:(tt + 1) * 128, :], in_=ot_t[:])

    nc.compile()
    return nc


def _stage1(tc, nc, P, h):
    """Sii row staging + S^T + exp + causal mask for head h."""
    p, hh = divmod(h, 2)
    hsl = slice(hh * 64, hh * 64 + 64)
    qt, kt = P["qt"], P["kt"]
    hw, hw2, smps, stps = P["hw"], P["hw2"], P["smps"], P["stps"]
    # Sii = sum_d QT*KT (raw logits incl 1/sqrt(d) from qt prescale)
    if hh == 0:
        qk = hw.tile([128, N], F32R, name="qk")
        nc.vector.tensor_tensor(out=qk[:], in0=qt[p].bitcast(F32)[:],
                                in1=kt[p].bitcast(F32)[:], op=ALU.mult)
        P["qk"][p] = qk
    qk = P["qk"][p]
    rst = hw2.tile([65, 4, 512], F32, name="rst")  # part 0 = D, part 64 = Sii
    P["rst"][h] = rst
    for nch in range(4):
        sl = slice(nch * 512, (nch + 1) * 512)
        ps = smps.tile([128, 512], F32, name="sm")
        nc.tensor.matmul(ps[0:1, :], P["ones_r"][hsl, :], qk[hsl, sl],
                         start=True, stop=True, tile_position=(hh * 64, 0))
        nc.scalar.copy(out=rst[64:65, nch, :], in_=ps[0:1, :])
    et = P["etp"].tile([128, ET_TOT], BF16, name="et")
    P["et"][h] = et
    for jt in range(NT):
        width = N - 128 * jt
        for ch in range((width + 1023) // 1024):
            cw = min(1024, width - ch * 1024)
            ps = stps.tile([128, 1024], F32, name="stp")
            for sub in range((cw + 511) // 512):
                scw = min(512, cw - sub * 512)
                i0 = 128 * jt + ch * 1024 + sub * 512
                nc.tensor.matmul(ps[:, sub * 512:sub * 512 + scw],
                                 kt[p][hsl, 128 * jt:128 * (jt + 1)],
                                 qt[p][hsl, i0:i0 + scw], start=True, stop=True,
                                 tile_position=(hh * 64, 0))
            nc.scalar.activation(
                out=et[:, ET_OFF[jt] + ch * 1024:ET_OFF[jt] + ch * 1024 + cw],
                in_=ps[:, 0:cw], func=AF.Exp)
        nc.gpsimd.affine_select(
            out=et[:, ET_OFF[jt]:ET_OFF[jt] + 128],
            in_=et[:, ET_OFF[jt]:ET_OFF[jt] + 128],
            compare_op=ALU.is_ge, fill=0.0, base=0,
            pattern=[[1, 128]], channel_multiplier=-1)


def _passA(tc, nc, P, h, evd):
    """[V_h|1]^T E^T -> EV rows into evd pair tile + D row into rst."""
    p, hh = divmod(h, 2)
    et = P["et"][h]
    rst = P["rst"][h]
    vt1 = P["vt1"]
    for ich in range(4):
        i0 = ich * 512
        ps = P["p3ps"].tile([128, 512], F32, name="psa")
        njt = min(NT, (ich + 1) * 4)
        for jt in range(njt):
            rel = max(0, 128 * jt - i0)
            ecol = ET_OFF[jt] + (i0 + rel - 128 * jt)
            nc.tensor.matmul(ps[0:65, rel:512], vt1[:, jt, h, :],
                             et[:, ecol:ecol + (512 - rel)],
                             start=(jt == 0), stop=(jt == njt - 1))
        nc.scalar.copy(out=evd[hh * 64:hh * 64 + 64, i0:i0 + 512], in_=ps[0:64, :])
        nc.scalar.copy(out=rst[0:1, ich, :], in_=ps[64:65, :])


def _scalars(tc, nc, P, h, bc_a, bc_b):
    """Column-layout per-token scalars via DRAM round-trip + broadcast DMAs."""
    p, hh = divmod(h, 2)
    hsl = slice(hh * 64, hh * 64 + 64)
    hw2 = P["hw2"]
    rst = P["rst"].pop(h)
    rows_d, rows_qf, rows_qb = P["rows_d"], P["rows_qf"], P["rows_qb"]
    # rows to DRAM ([2, N] f32), read back [token%128, tile] columns
    nc.sync.dma_start(out=rows_d[h][0], in_=rst[0:1].rearrange("r c w -> r (c w)"))
    nc.sync.dma_start(out=rows_d[h][1], in_=rst[64:65].rearrange("r c w -> r (c w)"))
    cols = hw2.tile([128, 2, NT], F32, name="cols")
    nc.sync.dma_start(out=cols[:], in_=bass.AP(
        tensor=rows_d.tensor, offset=rows_d.offset + h * 2 * N,
        ap=[[1, 128], [N, 2], [128, NT]]))
    # eii = exp(Sii); invd = 1/D; a = 1/(2D - eii); b = (D - eii) * a
    ecol = hw2.tile([128, NT], F32, name="ecol")
    nc.scalar.activation(out=ecol[:], in_=cols[:, 1, :], func=AF.Exp)
    qinvd = hw2.tile([128, NT], F32, name="qinvd")
    nc.vector.reciprocal(out=qinvd[:], in_=cols[:, 0, :])
    dent = hw2.tile([128, NT], F32, name="dent")
    nc.vector.tensor_scalar_mul(out=dent[:], in0=cols[:, 0, :], scalar1=2.0)
    nc.vector.tensor_tensor(out=dent[:], in0=dent[:], in1=ecol[:], op=ALU.subtract)
    qa = hw2.tile([128, NT], F32, name="qa")
    nc.vector.reciprocal(out=qa[:], in_=dent[:])
    dme = hw2.tile([128, NT], F32, name="dme")
    nc.vector.tensor_tensor(out=dme[:], in0=cols[:, 0, :], in1=ecol[:], op=ALU.subtract)
    qab = hw2.tile([128, 2, NT], BF16, name="qab")
    nc.vector.tensor_copy(out=qab[:, 0, :], in_=qa[:])
    nc.vector.tensor_tensor(out=qab[:, 1, :], in0=dme[:], in1=qa[:], op=ALU.mult)
    if P["has_gnn_b"]:
        # da = D * a (for the b1 ⊗ (D a) correction)
        da = hw2.tile([128, NT], BF16, name="da")
        nc.vector.tensor_tensor(out=da[:], in0=cols[:, 0, :], in1=qa[:], op=ALU.mult)
        nc.sync.dma_start(out=bass.AP(
            tensor=P["rows_da"].tensor, offset=P["rows_da"].offset + h * N,
            ap=[[1, 128], [128, NT]]), in_=da[:])
    # write rows back (row-major DRAM), then step-0 broadcast reads
    nc.sync.dma_start(out=bass.AP(
        tensor=rows_qf.tensor, offset=rows_qf.offset + h * N,
        ap=[[1, 128], [128, NT]]), in_=qinvd[:])
    nc.sync.dma_start(out=bass.AP(
        tensor=rows_qb.tensor, offset=rows_qb.offset + h * 2 * N,
        ap=[[1, 128], [N, 2], [128, NT]]), in_=qab[:])
    nc.scalar.dma_start(out=bc_a[hsl, :], in_=bass.AP(
        tensor=rows_qb.tensor, offset=rows_qb.offset + h * 2 * N, ap=[[0, 64], [1, N]]))
    nc.sync.dma_start(out=bc_b[hsl, :], in_=bass.AP(
        tensor=rows_qb.tensor, offset=rows_qb.offset + h * 2 * N + N, ap=[[0, 64], [1, N]]))


def _pair(tc, nc, P, p, cc_in, cc_out, groups):
    """Process head pair p (heads a=2p, b=2p+1) with pair-packed postproc."""
    a, b = 2 * p, 2 * p + 1
    hw, hw2, smps, p3ps = P["hw"], P["hw2"], P["smps"], P["p3ps"]
    ident_b = P["ident_b"]
    vt1 = P["vt1"]

    evd = hw.tile([128, N], BF16, name="evd")
    bc_a = hw.tile([128, N], BF16, name="bc_a")
    bc_b = hw.tile([128, N], BF16, name="bc_b")

    # --- stage A: pass A + per-token scalars for both heads ---
    _passA(tc, nc, P, a, evd)
    _scalars(tc, nc, P, a, bc_a, bc_b)
    _passA(tc, nc, P, b, evd)
    _scalars(tc, nc, P, b, bc_a, bc_b)

    yield

    # --- stage B: GNN layer 1 (pair-packed) ---
    # V^T pair tile via PE transposes
    vT = hw.tile([128, N], BF16, name="vT")
    for h in (a, b):
        hh = h % 2
        for quad in range(4):
            ps = smps.tile([128, 512], BF16, name="sm")
            for q in range(4):
                it = quad * 4 + q
                nc.tensor.transpose(ps[0:64, q * 128:(q + 1) * 128],
                                    vt1[:, it, h, 0:64], ident_b[:])
            nc.scalar.copy(out=vT[hh * 64:hh * 64 + 64, quad * 512:(quad + 1) * 512],
                           in_=ps[0:64, :])
    # w = (W1^T V^T + b1) * b + V^T
    wt = hw.tile([128, N], BF16, name="wt")
    for ch in range(4):
        sl = slice(ch * 512, (ch + 1) * 512)
        ps = smps.tile([128, 512], F32, name="sm")
        nc.tensor.matmul(ps[:], P["w1bd"][:], vT[:, sl], start=True, stop=True)
        if P["has_gnn_b"]:
            nc.vector.tensor_scalar_add(out=ps[:], in0=ps[:], scalar1=P["gnb_sb"][:, 0:1])
        nc.vector.tensor_tensor(out=wt[:, sl], in0=ps[:], in1=bc_b[:, sl], op=ALU.mult)
    nc.vector.tensor_tensor(out=wt[:], in0=wt[:], in1=vT[:], op=ALU.add)
    # t = (W1^T EV^T) * a + w ; f2 = gelu(t)
    tt = hw.tile([128, N], BF16, name="tt")
    for ch in range(4):
        sl = slice(ch * 512, (ch + 1) * 512)
        ps = smps.tile([128, 512], F32, name="sm")
        nc.tensor.matmul(ps[:], P["w1bd"][:], evd[:, sl], start=True, stop=True)
        nc.vector.tensor_tensor(out=tt[:, sl], in0=ps[:], in1=bc_a[:, sl], op=ALU.mult)
    if P["has_gnn_b"]:
        dab = hw.tile([128, N], BF16, name="dab")
        for hh2 in range(2):
            h2_ = 2 * p + hh2
            nc.sync.dma_start(out=dab[hh2 * 64:hh2 * 64 + 64, :], in_=bass.AP(
                tensor=P["rows_da"].tensor, offset=P["rows_da"].offset + h2_ * N,
                ap=[[0, 64], [1, N]]))
        nc.vector.tensor_scalar_mul(out=dab[:], in0=dab[:], scalar1=P["gnb_sb"][:, 0:1])
        nc.vector.tensor_tensor(out=tt[:], in0=tt[:], in1=dab[:], op=ALU.add)
    nc.vector.tensor_tensor(out=tt[:], in0=tt[:], in1=wt[:], op=ALU.add)
    f2 = hw.tile([128, N], BF16, name="f2")
    nc.scalar.activation(out=f2[:], in_=tt[:], func=AF.Gelu)
    # h2 = W2^T f2 (+b2), pair tile; token-layout copies per head
    h2p = hw.tile([128, N], BF16, name="h2p")
    for ch in range(4):
        sl = slice(ch * 512, (ch + 1) * 512)
        ps = smps.tile([128, 512], F32, name="sm")
        nc.tensor.matmul(ps[:], P["w2bd"][:], f2[:, sl], start=True, stop=True)
        if P["has_gnn_b"]:
            nc.vector.tensor_scalar_add(out=ps[:], in0=ps[:], scalar1=P["gnb_sb"][:, 1:2])
        nc.scalar.copy(out=h2p[:, sl], in_=ps[:])
    h2tok = {}
    for h in (a, b):
        hh = h % 2
        hsl = slice(hh * 64, hh * 64 + 64)
        ht = hw2.tile([128, NT, 64], BF16, name="h2tok")
        h2tok[h] = ht
        for quad in range(4):
            ps = smps.tile([128, 512], BF16, name="sm")
            for q in range(4):
                it = quad * 4 + q
                nc.tensor.transpose(ps[:, q * 64:(q + 1) * 64],
                                    h2p[hsl, it * 128:(it + 1) * 128],
                                    ident_b[hsl, hsl])
            nc.scalar.copy(out=ht[:, quad * 4:(quad + 1) * 4, :],
                           in_=ps[:, 0:256].rearrange("p (t d) -> p t d", t=4))

    yield

    # --- stage C: pass B + combine + out + AllToAll ---
    ebp = hw.tile([128, N], BF16, name="vT")      # reuse vT slot
    for h in (a, b):
        hh = h % 2
        et = P["et"].pop(h)
        ht = h2tok[h]
        for ich in range(4):
            i0 = ich * 512
            ps = p3ps.tile([128, 512], F32, name="psa")
            njt = min(NT, (ich + 1) * 4)
            for jt in range(njt):
                rel = max(0, 128 * jt - i0)
                ecol = ET_OFF[jt] + (i0 + rel - 128 * jt)
                nc.tensor.matmul(ps[0:64, rel:512], ht[:, jt, :],
                                 et[:, ecol:ecol + (512 - rel)],
                                 start=(jt == 0), stop=(jt == njt - 1))
            nc.scalar.copy(out=ebp[hh * 64:hh * 64 + 64, i0:i0 + 512], in_=ps[0:64, :])
    # t4 = ebp * a + (h2p * b + f2) ; f3 = gelu(t4)
    t4 = hw.tile([128, N], BF16, name="wt")       # reuse wt slot
    nc.vector.tensor_tensor(out=t4[:], in0=ebp[:], in1=bc_a[:], op=ALU.mult)
    u = hw.tile([128, N], BF16, name="tt")        # reuse tt slot
    nc.vector.tensor_tensor(out=u[:], in0=h2p[:], in1=bc_b[:], op=ALU.mult)
    nc.vector.tensor_tensor(out=u[:], in0=u[:], in1=f2[:], op=ALU.add)
    nc.vector.tensor_tensor(out=t4[:], in0=t4[:], in1=u[:], op=ALU.add)
    f3 = hw.tile([128, N], BF16, name="tt")       # reuse tt slot (u is dead)
    nc.scalar.activation(out=f3[:], in_=t4[:], func=AF.Gelu)
    # invd pair rows -> PE rank-1 broadcast; out = lin_w^T f3 (+lin_b) + EV*invd
    rowp = hw.tile([1, 2, N], F32, name="rowpair")
    for hh in range(2):
        h = 2 * p + hh
        nc.sync.dma_start(out=rowp[0:1, hh, :], in_=bass.AP(
            tensor=P["rows_qf"].tensor, offset=P["rows_qf"].offset + h * N,
            ap=[[1, 1], [1, N]]))
    ocast = hw.tile([128, N], BF16, name="ocast")
    for ch in range(4):
        sl = slice(ch * 512, (ch + 1) * 512)
        psl = smps.tile([128, 512], F32, name="sm")
        nc.tensor.matmul(psl[:], P["linbd"][:], f3[:, sl], start=True, stop=True)
        if P["has_lin_b"]:
            nc.vector.tensor_scalar_add(out=psl[:], in0=psl[:], scalar1=P["linb_sb"][:])
        psb = smps.tile([128, 512], F32, name="sm")
        nc.tensor.matmul(psb[:], P["ones2a"][:], rowp.bitcast(F32R)[0:1, 0, sl],
                         start=True, stop=False)
        nc.tensor.matmul(psb[:], P["ones2b"][:], rowp.bitcast(F32R)[0:1, 1, sl],
                         start=False, stop=True)
        at = hw2.tile([128, 512], F32, name="at")
        nc.vector.tensor_tensor(out=at[:], in0=evd[:, sl], in1=psb[:], op=ALU.mult)
        nc.vector.tensor_tensor(out=ocast[:, sl], in0=psl[:], in1=at[:], op=ALU.add)
    for half in range(2):
        nc.sync.dma_start(
            out=cc_in[p][half * 512:(half + 1) * 512].rearrange("(g p) t -> p g t", p=128),
            in_=ocast[:])
    nc.gpsimd.collective_compute(
        "AllToAll", ALU.bypass, replica_groups=groups,
        ins=[cc_in[p]], outs=[cc_out[p]])

    yield


# ---------------------------------------------------------------------------

_PROGRAM_CACHE = {}


def kernel(x, gamma_in, Wq, Wk, Wv, gnn_w, gnn_b, lin_w, lin_b, Wo, gamma_out):
    import ml_dtypes
    x = np.asarray(x, dtype=np.float32)
    gamma_in = np.asarray(gamma_in, dtype=np.float32)
    Wq = np.asarray(Wq, dtype=np.float32)
    Wk = np.asarray(Wk, dtype=np.float32)
    Wv = np.asarray(Wv, dtype=np.float32)
    gnn_w = np.asarray(gnn_w, dtype=np.float32)
    gnn_b = np.asarray(gnn_b, dtype=np.float32)
    lin_w = np.asarray(lin_w, dtype=np.float32)
    lin_b = np.asarray(lin_b, dtype=np.float32)
    Wo = np.asarray(Wo, dtype=np.float32)
    gamma_out = np.asarray(gamma_out, dtype=np.float32)

    key = (bool(np.any(gnn_b)), bool(np.any(lin_b)),
           bool(np.all(gamma_in == 1.0)), bool(np.all(gamma_out == 1.0)))
    if key not in _PROGRAM_CACHE:
        _PROGRAM_CACHE[key] = build_program(key[0], key[1], key[2], key[3])
    nc = _PROGRAM_CACHE[key]

    wo_bf = np.ascontiguousarray(Wo).astype(ml_dtypes.bfloat16)
    in_maps = []
    for c in range(8):
        b, g = divmod(c, 4)
        cs = slice(g * 256, (g + 1) * 256)
        in_maps.append(dict(
            x_in=np.ascontiguousarray(x[b]),
            wq_in=np.ascontiguousarray(Wq[:, cs]),
            wk_in=np.ascontiguousarray(Wk[:, cs]),
            wv_in=np.ascontiguousarray(Wv[:, cs]),
            wo_in=wo_bf,
            gin_in=gamma_in[None, :],
            gout_in=gamma_out[None, :],
            gnw_in=gnn_w.astype(ml_dtypes.bfloat16),
            gnb_in=gnn_b,
            linw_in=lin_w.astype(ml_dtypes.bfloat16),
            linb_in=lin_b[None, :],
            bmask_in=np.array([[1.0, 0.0]] if c < 4 else [[0.0, 1.0]],
                              dtype=np.float32),
        ))
    trace = bool(int(os.environ.get("KERNEL_TRACE", "0")))
    res = run_bass_kernel_spmd(nc, in_maps, list(range(8)), trace=trace)
    if trace:
        kernel.last_exec_time_ns = res.exec_time_ns

    out = np.empty((B, N, DIM), dtype=np.float32)
    for c in range(8):
        b, g = divmod(c, 4)
        yc = res.results[c]["y_out"]
        out[b, 512 * g:512 * (g + 1), :] = yc
    return out
